# revision 10
# baseline (speedup 1.0000x reference)
"""Trainium2 Bass kernel for nn_Decoder_6055903887927 (gnn_message_passing).

Math (per irrep i, d_i in (1,3,5)):
  h = silu(silu(inv @ w1 + b1) @ w2 + b2)
  r2f = (h @ w3 + b3) * 1/sqrt(RBF)            # (A, RBF, F)
  sparse[t,f] += sum_{n,d,r} sph_i[n,t,d] * feat_i[n,f,d]
                             * rbf[n,t,r] * r2f[n,r,f]
  out[idx[t]] += sparse[t]                     # scatter-add into (N, F)

Strategy: CP-expansion of the einsum into one big matmul with contraction
axis K = (d, r, n) of size 9*16*128 = 18432:
  sparse^T[f, t] = sum_K W[K, f] * P[K, t]
  W[(d,r,n), f] = feat_d[n, f] * r2f[n, r, f]     (elementwise build)
  P[(d,r,n), t] = sph_d[n, t] * rbf_r[n, t]       (elementwise build)
Both builds have n on partitions -> matmul contracts partitions directly.

Sharding: split the TARGET axis t (T=2048) across 8 cores (TL=256 each).
The einsum reduces over (n, d, r), not t, so each core's result rows are
complete - no all-reduce. One small AllGather (128KB/rank) replicates the
(T, F) result so each core can scatter the rows it owns (output sharded
by N-range, NR=8192 rows per core). Duplicate target indices are merged
with statically-emitted adds (the program is compiled per idx), and the
scatter is one bounds-checked indirect DMA per 128 rows: offsets out of
[0, NR) are skipped, which both filters ownership and skips dup non-leaders.
"""

import sys

sys.path.insert(0, "/opt/trn_rl_repo")

import numpy as np

import concourse.bass as bass
import concourse.mybir as mybir
from concourse import bacc, tile
from concourse.masks import make_identity

A, T, NGRID, RBF, F = 128, 2048, 65536, 16, 128
DS = (1, 3, 5)
NDP = sum(DS)  # 9 d-planes
NCORES = 8
TL = T // NCORES  # 256 targets per core
NR = NGRID // NCORES  # 8192 output rows per core
OOB = 1 << 30  # offset value that fails bounds_check -> row skipped

F32 = mybir.dt.float32
BF16 = mybir.dt.bfloat16
F16 = mybir.dt.float16
I32 = mybir.dt.int32
MD_MAP = {"f32": F32, "bf16": BF16, "f16": F16}

# "f32" or "bf16": dtype of the P/W elementwise builds + the big matmul.
MAIN_DTYPE = "f16"

_CACHE: dict = {}


def _build_program(dup_pairs, b3_nonzero, main_dtype, repeats=1):
    """Build + compile the SPMD program (shared by all 8 cores)."""
    md = MD_MAP[main_dtype]
    nc = bacc.Bacc(
        "TRN2", target_bir_lowering=False, debug=False, num_devices=NCORES
    )

    invT_h = nc.dram_tensor("invT", [F, A], F32, kind="ExternalInput")
    w1_h = nc.dram_tensor("w1", [3, F, F], F32, kind="ExternalInput")
    b1_h = nc.dram_tensor("b1", [3, F, 1], F32, kind="ExternalInput")
    w2_h = nc.dram_tensor("w2", [3, F, F], F32, kind="ExternalInput")
    b2_h = nc.dram_tensor("b2", [3, F, 1], F32, kind="ExternalInput")
    w3_h = nc.dram_tensor("w3", [3, F, RBF * F], md, kind="ExternalInput")
    if b3_nonzero:
        b3_h = nc.dram_tensor("b3", [3, 1, RBF * F], md, kind="ExternalInput")
    featp_h = nc.dram_tensor("featp", [NDP, A, F], md, kind="ExternalInput")
    sphp_h = nc.dram_tensor("sphp", [NDP, A, TL], md, kind="ExternalInput")
    rbfp_h = nc.dram_tensor("rbfp", [RBF, A, TL], md, kind="ExternalInput")
    offs_h = nc.dram_tensor("offs", [128, T // 128], I32, kind="ExternalInput")
    out_h = nc.dram_tensor("out", [NR, F], F32, kind="ExternalOutput")

    with tile.TileContext(nc) as tc:
        with (
            tc.tile_pool(name="const", bufs=1) as const,
            tc.tile_pool(name="mlp", bufs=2) as mlp,
            tc.tile_pool(name="work", bufs=3) as work,
            tc.tile_pool(name="psacc", bufs=1, space="PSUM") as psacc,
            tc.tile_pool(name="pssm", bufs=2, space="PSUM") as pssm,
            tc.tile_pool(name="psbig", bufs=1, space="PSUM") as psbig,
            tc.tile_pool(name="dram", bufs=1, space="DRAM") as dram,
        ):
            ident = const.tile([128, 128], F32, tag="ident")
            make_identity(nc, ident[:])

            invT_t = const.tile([F, A], F32, tag="invT")
            nc.sync.dma_start(invT_t[:], invT_h[:])
            w1_t, b1_t, w2_t, b2_t, w3_t, b3_t = [], [], [], [], [], []
            for i in range(3):
                w1_t.append(const.tile([F, F], F32, tag=f"w1_{i}", name=f"w1_{i}"))
                nc.sync.dma_start(w1_t[i][:], w1_h[i])
                b1_t.append(const.tile([F, 1], F32, tag=f"b1_{i}", name=f"b1_{i}"))
                nc.sync.dma_start(b1_t[i][:], b1_h[i])
                w2_t.append(const.tile([F, F], F32, tag=f"w2_{i}", name=f"w2_{i}"))
                nc.sync.dma_start(w2_t[i][:], w2_h[i])
                b2_t.append(const.tile([F, 1], F32, tag=f"b2_{i}", name=f"b2_{i}"))
                nc.sync.dma_start(b2_t[i][:], b2_h[i])
                w3_t.append(const.tile([F, RBF * F], md, tag=f"w3_{i}", name=f"w3_{i}"))
                nc.sync.dma_start(w3_t[i][:], w3_h[i])
                if b3_nonzero:
                    b3_t.append(const.tile([1, RBF * F], md, tag=f"b3_{i}", name=f"b3_{i}"))
                    nc.sync.dma_start(b3_t[i][:], b3_h[i])
            if b3_nonzero:
                ones_t = const.tile([1, A], md, tag="ones")
                nc.gpsimd.memset(ones_t[:], 1.0)

            featp_t = const.tile([A, NDP * F], md, tag="featp")
            nc.sync.dma_start(
                featp_t[:].rearrange("p (n f) -> p n f", n=NDP),
                featp_h[:].transpose([1, 0, 2]),
            )
            sph_t = const.tile([A, NDP * TL], md, tag="sph")
            nc.sync.dma_start(
                sph_t[:].rearrange("p (n t) -> p n t", n=NDP),
                sphp_h[:].transpose([1, 0, 2]),
            )
            rbf_t = const.tile([A, RBF * TL], md, tag="rbf")
            nc.sync.dma_start(
                rbf_t[:].rearrange("p (r t) -> p r t", r=RBF),
                rbfp_h[:].transpose([1, 0, 2]),
            )
            offs_t = const.tile([128, T // 128], I32, tag="offs")
            nc.sync.dma_start(offs_t[:], offs_h[:])

            for _rep in range(repeats):
                # ---- MLP: r2f[i] = silu(silu(inv@w1+b1)@w2+b2) @ w3 ----
                # (w3 pre-scaled by 1/sqrt(RBF) on host)
                r2f_t = []
                for i in range(3):
                    h1p = pssm.tile([F, A], F32, tag="hsm", bufs=2, name="h1p")
                    nc.tensor.matmul(
                        h1p[:], w1_t[i][:], invT_t[:], start=True, stop=True
                    )
                    h1 = mlp.tile([F, A], F32, tag="h1")
                    nc.scalar.activation(
                        h1[:], h1p[:], mybir.ActivationFunctionType.Silu,
                        bias=b1_t[i][:],
                    )
                    h2p = pssm.tile([F, A], F32, tag="hsm", bufs=2, name="h2p")
                    nc.tensor.matmul(
                        h2p[:], w2_t[i][:], h1[:], start=True, stop=True
                    )
                    h2 = mlp.tile([F, A], md, tag="h2")
                    nc.scalar.activation(
                        h2[:], h2p[:], mybir.ActivationFunctionType.Silu,
                        bias=b2_t[i][:],
                    )
                    r2f = mlp.tile([A, RBF * F], md, tag=f"r2f_{i}")
                    for jh in range(2):
                        r2fp = psbig.tile([A, 1024], F32, tag="r2fp")
                        for j in range(2):
                            sl = slice(jh * 1024 + j * 512, jh * 1024 + (j + 1) * 512)
                            psl = slice(j * 512, (j + 1) * 512)
                            nc.tensor.matmul(
                                r2fp[:, psl], h2[:], w3_t[i][:, sl],
                                start=True, stop=not b3_nonzero,
                            )
                            if b3_nonzero:
                                nc.tensor.matmul(
                                    r2fp[:, psl], ones_t[:], b3_t[i][:, sl],
                                    start=False, stop=True,
                                )
                        nc.scalar.activation(
                            r2f[:, jh * 1024:(jh + 1) * 1024], r2fp[:],
                            mybir.ActivationFunctionType.Copy,
                        )
                    r2f_t.append(r2f)

                # ---- main contraction: acc[f, t] = sum_K W[K,f] P[K,t] ----
                acc = psacc.tile([F, TL], F32, tag="acc")
                nmm = NDP * RBF
                it = 0
                for i in range(3):
                    for d in range(DS[i]):
                        dg = sum(DS[:i]) + d
                        pb = work.tile([A, RBF, TL], md, tag="pb")
                        nc.vector.tensor_mul(
                            pb[:],
                            rbf_t[:].rearrange("p (r t) -> p r t", r=RBF),
                            sph_t[:, dg * TL:(dg + 1) * TL]
                            .unsqueeze(1).broadcast_to([A, RBF, TL]),
                        )
                        wb = work.tile([A, RBF, F], md, tag="wb")
                        nc.gpsimd.tensor_mul(
                            wb[:],
                            r2f_t[i][:].rearrange("p (r f) -> p r f", r=RBF),
                            featp_t[:, dg * F:(dg + 1) * F]
                            .unsqueeze(1).broadcast_to([A, RBF, F]),
                        )
                        for r in range(RBF):
                            nc.tensor.matmul(
                                acc[:], wb[:, r, :], pb[:, r, :],
                                start=(it == 0), stop=(it == nmm - 1),
                            )
                            it += 1

                # ---- AllGather the [f, t]-layout partials ----
                accs = work.tile([F, TL], F32, tag="accs")
                nc.scalar.activation(
                    accs[:], acc[:], mybir.ActivationFunctionType.Copy
                )
                agin = dram.tile([F, TL], F32)
                nc.gpsimd.dma_start(agin[:], accs[:])
                agout = dram.tile([NCORES * F, TL], F32)
                nc.gpsimd.collective_compute(
                    "AllGather",
                    mybir.AluOpType.bypass,
                    replica_groups=[list(range(NCORES))],
                    ins=[agin[:].opt()],
                    outs=[agout[:].opt()],
                )

                # full (T, F) result as columns: cols[f, t]
                cols = const.tile([F, T], F32, tag="cols")
                nc.sync.dma_start(
                    cols[:].rearrange("p (c t) -> p c t", c=NCORES),
                    agout[:].rearrange("(c f) t -> f c t", c=NCORES),
                )
                # merge duplicate targets (free-axis column adds are legal)
                for lt, dt_ in dup_pairs:
                    nc.vector.tensor_add(
                        cols[:, lt:lt + 1],
                        cols[:, lt:lt + 1],
                        cols[:, dt_:dt_ + 1],
                    )
                # transpose to row-major and scatter owned rows
                for k in range(T // 128):
                    trp = pssm.tile([128, 128], F32, tag="trp", bufs=2)
                    nc.tensor.transpose(
                        trp[:], cols[:, k * 128:(k + 1) * 128], ident[:]
                    )
                    rl = work.tile([128, 128], F32, tag="rl")
                    nc.scalar.activation(
                        rl[:], trp[:], mybir.ActivationFunctionType.Copy
                    )
                    nc.gpsimd.indirect_dma_start(
                        out=out_h[:],
                        out_offset=bass.IndirectOffsetOnAxis(
                            ap=offs_t[:, k:k + 1], axis=0
                        ),
                        in_=rl[:],
                        in_offset=None,
                        bounds_check=NR - 1,
                        oob_is_err=False,
                    )

    nc.compile()
    return nc


def _prep(inputs, main_dtype):
    """Host-side input prep -> (per-core in_maps, dup_pairs, b3_nonzero)."""
    if main_dtype == "f32":
        md = np.float32
    elif main_dtype == "f16":
        md = np.float16
    else:
        import ml_dtypes

        md = np.dtype(ml_dtypes.bfloat16)
    f0 = np.asarray(inputs["feat0"], np.float32)
    inv_rbf = np.float32(1.0 / np.sqrt(RBF))

    invT = np.ascontiguousarray(f0[:, :, 0].T)
    w1 = np.ascontiguousarray(np.asarray(inputs["mlp_w1"], np.float32))
    b1 = np.asarray(inputs["mlp_b1"], np.float32).reshape(3, F, 1).copy()
    w2 = np.ascontiguousarray(np.asarray(inputs["mlp_w2"], np.float32))
    b2 = np.asarray(inputs["mlp_b2"], np.float32).reshape(3, F, 1).copy()
    w3 = np.ascontiguousarray(np.asarray(inputs["mlp_w3"], np.float32) * inv_rbf).astype(md)
    b3 = (np.asarray(inputs["mlp_b3"], np.float32) * inv_rbf)
    b3_nonzero = bool(np.any(b3))
    b3 = b3.reshape(3, 1, RBF * F).astype(md)

    featp = np.concatenate(
        [
            np.asarray(inputs[f"feat{i}"], np.float32).transpose(2, 0, 1)
            for i in range(3)
        ],
        axis=0,
    ).astype(md)  # (9, A, F)
    sphp = np.concatenate(
        [
            np.asarray(inputs[f"sph{i}"], np.float32).transpose(2, 0, 1)
            for i in range(3)
        ],
        axis=0,
    ).astype(md)  # (9, A, T)
    rbfp = (
        np.asarray(inputs["radial_basis_vals"], np.float32)
        .transpose(2, 0, 1)
        .astype(md)
    )  # (RBF, A, T)

    idx = np.asarray(inputs["truncated_idx"]).astype(np.int64)
    first: dict = {}
    dup_pairs = []
    for t, v in enumerate(idx.tolist()):
        if v in first:
            dup_pairs.append((first[v], t))
        else:
            first[v] = t

    in_maps = []
    for c in range(NCORES):
        off = np.full(T, OOB, np.int32)
        lo, hi = c * NR, (c + 1) * NR
        for v, lt in first.items():
            if lo <= v < hi:
                off[lt] = v - lo
        offs2d = np.ascontiguousarray(off.reshape(T // 128, 128).T)
        ts = slice(c * TL, (c + 1) * TL)
        m = {
            "invT": invT,
            "w1": w1, "b1": b1, "w2": w2, "b2": b2, "w3": w3,
            "featp": featp,
            "sphp": np.ascontiguousarray(sphp[:, :, ts]),
            "rbfp": np.ascontiguousarray(rbfp[:, :, ts]),
            "offs": offs2d,
        }
        if b3_nonzero:
            m["b3"] = b3
        in_maps.append(m)
    return in_maps, tuple(dup_pairs), b3_nonzero


def _get_runner(dup_pairs, b3_nonzero, main_dtype, repeats=1):
    key = (dup_pairs, b3_nonzero, main_dtype, repeats)
    if key not in _CACHE:
        nc = _build_program(dup_pairs, b3_nonzero, main_dtype, repeats)
        _CACHE[key] = nc
    return _CACHE[key]


def run_on_hw(in_maps, nc):
    from concourse import bass_utils

    res = bass_utils.run_bass_kernel_spmd(
        nc, in_maps, core_ids=list(range(NCORES))
    )
    return res.results


def kernel(**inputs) -> np.ndarray:
    in_maps, dup_pairs, b3_nonzero = _prep(inputs, MAIN_DTYPE)
    nc = _get_runner(dup_pairs, b3_nonzero, MAIN_DTYPE)
    results = run_on_hw(in_maps, nc)
    return np.concatenate(
        [results[c]["out"] for c in range(NCORES)], axis=0
    )


# revision 11
# speedup vs baseline: 1.1894x; 1.1894x over previous
"""Trainium2 Bass kernel for nn_Decoder_6055903887927 (gnn_message_passing).

Math (per irrep i, d_i in (1,3,5)):
  h = silu(silu(inv @ w1 + b1) @ w2 + b2)
  r2f = (h @ w3 + b3) * 1/sqrt(RBF)            # (A, RBF, F)
  sparse[t,f] += sum_{n,d,r} sph_i[n,t,d] * feat_i[n,f,d]
                             * rbf[n,t,r] * r2f[n,r,f]
  out[idx[t]] += sparse[t]                     # scatter-add into (N, F)

Strategy: CP-expansion of the einsum into one big matmul with contraction
axis K = (d, r, n) of size 9*16*128 = 18432:
  sparse^T[f, t] = sum_K W[K, f] * P[K, t]
  W[(d,r,n), f] = feat_d[n, f] * r2f[n, r, f]     (elementwise build)
  P[(d,r,n), t] = sph_d[n, t] * rbf_r[n, t]       (elementwise build)
Both builds have n on partitions -> matmul contracts partitions directly.

Sharding: split the TARGET axis t (T=2048) across 8 cores (TL=256 each).
The einsum reduces over (n, d, r), not t, so each core's result rows are
complete - no all-reduce. One small AllGather (128KB/rank) replicates the
(T, F) result so each core can scatter the rows it owns (output sharded
by N-range, NR=8192 rows per core). Duplicate target indices are merged
with statically-emitted adds (the program is compiled per idx), and the
scatter is one bounds-checked indirect DMA per 128 rows: offsets out of
[0, NR) are skipped, which both filters ownership and skips dup non-leaders.
"""

import sys

sys.path.insert(0, "/opt/trn_rl_repo")

import numpy as np

import concourse.bass as bass
import concourse.mybir as mybir
from concourse import bacc, tile
from concourse.masks import make_identity

A, T, NGRID, RBF, F = 128, 2048, 65536, 16, 128
DS = (1, 3, 5)
NDP = sum(DS)  # 9 d-planes
NCORES = 8
TL = T // NCORES  # 256 targets per core
NR = NGRID // NCORES  # 8192 output rows per core
OOB = 1 << 30  # offset value that fails bounds_check -> row skipped

F32 = mybir.dt.float32
BF16 = mybir.dt.bfloat16
F16 = mybir.dt.float16
I32 = mybir.dt.int32
MD_MAP = {"f32": F32, "bf16": BF16, "f16": F16}

# "f32" or "bf16": dtype of the P/W elementwise builds + the big matmul.
MAIN_DTYPE = "f16"

_CACHE: dict = {}


def _build_program(dup_pairs, b3_nonzero, main_dtype, repeats=1):
    """Build + compile the SPMD program (shared by all 8 cores)."""
    md = MD_MAP[main_dtype]
    nc = bacc.Bacc(
        "TRN2", target_bir_lowering=False, debug=False, num_devices=NCORES
    )

    invT_h = nc.dram_tensor("invT", [F, A], F32, kind="ExternalInput")
    w1_h = nc.dram_tensor("w1", [3, F, F], F32, kind="ExternalInput")
    b1_h = nc.dram_tensor("b1", [3, F, 1], F32, kind="ExternalInput")
    w2_h = nc.dram_tensor("w2", [3, F, F], F32, kind="ExternalInput")
    b2_h = nc.dram_tensor("b2", [3, F, 1], F32, kind="ExternalInput")
    w3_h = nc.dram_tensor("w3", [3, F, RBF * F], md, kind="ExternalInput")
    if b3_nonzero:
        b3_h = nc.dram_tensor("b3", [3, 1, RBF * F], md, kind="ExternalInput")
    featp_h = nc.dram_tensor("featp", [NDP, A, F], md, kind="ExternalInput")
    sphp_h = nc.dram_tensor("sphp", [NDP, A, TL], md, kind="ExternalInput")
    rbfp_h = nc.dram_tensor("rbfp", [RBF, A, TL], md, kind="ExternalInput")
    offs_h = nc.dram_tensor("offs", [128, T // 128], I32, kind="ExternalInput")
    out_h = nc.dram_tensor("out", [NR, F], F32, kind="ExternalOutput")

    with tile.TileContext(nc) as tc:
        with (
            tc.tile_pool(name="const", bufs=1) as const,
            tc.tile_pool(name="mlp", bufs=2) as mlp,
            tc.tile_pool(name="work", bufs=3) as work,
            tc.tile_pool(name="psacc", bufs=1, space="PSUM") as psacc,
            tc.tile_pool(name="pssm", bufs=2, space="PSUM") as pssm,
            tc.tile_pool(name="psbig", bufs=1, space="PSUM") as psbig,
            tc.tile_pool(name="dram", bufs=1, space="DRAM") as dram,
        ):
            ident = const.tile([128, 128], F32, tag="ident")
            make_identity(nc, ident[:])

            invT_t = const.tile([F, A], F32, tag="invT")
            nc.sync.dma_start(invT_t[:], invT_h[:])
            w1_t, b1_t, w2_t, b2_t, w3_t, b3_t = [], [], [], [], [], []
            for i in range(3):
                w1_t.append(const.tile([F, F], F32, tag=f"w1_{i}", name=f"w1_{i}"))
                nc.sync.dma_start(w1_t[i][:], w1_h[i])
                b1_t.append(const.tile([F, 1], F32, tag=f"b1_{i}", name=f"b1_{i}"))
                nc.sync.dma_start(b1_t[i][:], b1_h[i])
                w2_t.append(const.tile([F, F], F32, tag=f"w2_{i}", name=f"w2_{i}"))
                nc.sync.dma_start(w2_t[i][:], w2_h[i])
                b2_t.append(const.tile([F, 1], F32, tag=f"b2_{i}", name=f"b2_{i}"))
                nc.sync.dma_start(b2_t[i][:], b2_h[i])
                w3_t.append(const.tile([F, RBF * F], md, tag=f"w3_{i}", name=f"w3_{i}"))
                nc.sync.dma_start(w3_t[i][:], w3_h[i])
                if b3_nonzero:
                    b3_t.append(const.tile([1, RBF * F], md, tag=f"b3_{i}", name=f"b3_{i}"))
                    nc.sync.dma_start(b3_t[i][:], b3_h[i])
            if b3_nonzero:
                ones_t = const.tile([1, A], md, tag="ones")
                nc.gpsimd.memset(ones_t[:], 1.0)

            featp_t = const.tile([A, NDP * F], md, tag="featp")
            nc.sync.dma_start(
                featp_t[:].rearrange("p (n f) -> p n f", n=NDP),
                featp_h[:].transpose([1, 0, 2]),
            )
            sph_t = const.tile([A, NDP * TL], md, tag="sph")
            nc.sync.dma_start(
                sph_t[:].rearrange("p (n t) -> p n t", n=NDP),
                sphp_h[:].transpose([1, 0, 2]),
            )
            rbf_t = const.tile([A, RBF * TL], md, tag="rbf")
            nc.sync.dma_start(
                rbf_t[:].rearrange("p (r t) -> p r t", r=RBF),
                rbfp_h[:].transpose([1, 0, 2]),
            )
            offs_t = const.tile([128, T // 128], I32, tag="offs")
            nc.sync.dma_start(offs_t[:], offs_h[:])

            for _rep in range(repeats):
                # ---- MLP: r2f[i] = silu(silu(inv@w1+b1)@w2+b2) @ w3 ----
                # (w3 pre-scaled by 1/sqrt(RBF) on host)
                r2f_t = []
                for i in range(3):
                    h1p = pssm.tile([F, A], F32, tag="hsm", bufs=2, name="h1p")
                    nc.tensor.matmul(
                        h1p[:], w1_t[i][:], invT_t[:], start=True, stop=True
                    )
                    h1 = mlp.tile([F, A], F32, tag="h1")
                    nc.scalar.activation(
                        h1[:], h1p[:], mybir.ActivationFunctionType.Silu,
                        bias=b1_t[i][:],
                    )
                    h2p = pssm.tile([F, A], F32, tag="hsm", bufs=2, name="h2p")
                    nc.tensor.matmul(
                        h2p[:], w2_t[i][:], h1[:], start=True, stop=True
                    )
                    h2 = mlp.tile([F, A], md, tag="h2")
                    nc.scalar.activation(
                        h2[:], h2p[:], mybir.ActivationFunctionType.Silu,
                        bias=b2_t[i][:],
                    )
                    r2f = mlp.tile([A, RBF * F], md, tag=f"r2f_{i}")
                    for jh in range(2):
                        r2fp = psbig.tile([A, 1024], F32, tag="r2fp")
                        for j in range(2):
                            sl = slice(jh * 1024 + j * 512, jh * 1024 + (j + 1) * 512)
                            psl = slice(j * 512, (j + 1) * 512)
                            nc.tensor.matmul(
                                r2fp[:, psl], h2[:], w3_t[i][:, sl],
                                start=True, stop=not b3_nonzero,
                            )
                            if b3_nonzero:
                                nc.tensor.matmul(
                                    r2fp[:, psl], ones_t[:], b3_t[i][:, sl],
                                    start=False, stop=True,
                                )
                        nc.scalar.activation(
                            r2f[:, jh * 1024:(jh + 1) * 1024], r2fp[:],
                            mybir.ActivationFunctionType.Copy,
                        )
                    r2f_t.append(r2f)

                # ---- main contraction: acc[f, t] = sum_K W[K,f] P[K,t] ----
                acc = psacc.tile([F, TL], F32, tag="acc")
                nmm = NDP * RBF
                it = 0
                for i in range(3):
                    for d in range(DS[i]):
                        dg = sum(DS[:i]) + d
                        pb = work.tile([A, RBF, TL], md, tag="pb")
                        nc.vector.tensor_mul(
                            pb[:],
                            rbf_t[:].rearrange("p (r t) -> p r t", r=RBF),
                            sph_t[:, dg * TL:(dg + 1) * TL]
                            .unsqueeze(1).broadcast_to([A, RBF, TL]),
                        )
                        wb = work.tile([A, RBF, F], md, tag="wb")
                        nc.vector.tensor_mul(
                            wb[:],
                            r2f_t[i][:].rearrange("p (r f) -> p r f", r=RBF),
                            featp_t[:, dg * F:(dg + 1) * F]
                            .unsqueeze(1).broadcast_to([A, RBF, F]),
                        )
                        for r in range(RBF):
                            nc.tensor.matmul(
                                acc[:], wb[:, r, :], pb[:, r, :],
                                start=(it == 0), stop=(it == nmm - 1),
                            )
                            it += 1

                # ---- AllGather the [f, t]-layout partials ----
                accs = work.tile([F, TL], F32, tag="accs")
                nc.scalar.activation(
                    accs[:], acc[:], mybir.ActivationFunctionType.Copy
                )
                agin = dram.tile([F, TL], F32)
                nc.gpsimd.dma_start(agin[:], accs[:])
                agout = dram.tile([NCORES * F, TL], F32)
                nc.gpsimd.collective_compute(
                    "AllGather",
                    mybir.AluOpType.bypass,
                    replica_groups=[list(range(NCORES))],
                    ins=[agin[:].opt()],
                    outs=[agout[:].opt()],
                )

                # full (T, F) result as columns: cols[f, t]
                cols = const.tile([F, T], F32, tag="cols")
                nc.sync.dma_start(
                    cols[:].rearrange("p (c t) -> p c t", c=NCORES),
                    agout[:].rearrange("(c f) t -> f c t", c=NCORES),
                )
                # merge duplicate targets (free-axis column adds are legal)
                for lt, dt_ in dup_pairs:
                    nc.vector.tensor_add(
                        cols[:, lt:lt + 1],
                        cols[:, lt:lt + 1],
                        cols[:, dt_:dt_ + 1],
                    )
                # transpose to row-major and scatter owned rows
                for k in range(T // 128):
                    trp = pssm.tile([128, 128], F32, tag="trp", bufs=2)
                    nc.tensor.transpose(
                        trp[:], cols[:, k * 128:(k + 1) * 128], ident[:]
                    )
                    rl = work.tile([128, 128], F32, tag="rl")
                    nc.scalar.activation(
                        rl[:], trp[:], mybir.ActivationFunctionType.Copy
                    )
                    nc.gpsimd.indirect_dma_start(
                        out=out_h[:],
                        out_offset=bass.IndirectOffsetOnAxis(
                            ap=offs_t[:, k:k + 1], axis=0
                        ),
                        in_=rl[:],
                        in_offset=None,
                        bounds_check=NR - 1,
                        oob_is_err=False,
                    )

    nc.compile()
    return nc


def _prep(inputs, main_dtype):
    """Host-side input prep -> (per-core in_maps, dup_pairs, b3_nonzero)."""
    if main_dtype == "f32":
        md = np.float32
    elif main_dtype == "f16":
        md = np.float16
    else:
        import ml_dtypes

        md = np.dtype(ml_dtypes.bfloat16)
    f0 = np.asarray(inputs["feat0"], np.float32)
    inv_rbf = np.float32(1.0 / np.sqrt(RBF))

    invT = np.ascontiguousarray(f0[:, :, 0].T)
    w1 = np.ascontiguousarray(np.asarray(inputs["mlp_w1"], np.float32))
    b1 = np.asarray(inputs["mlp_b1"], np.float32).reshape(3, F, 1).copy()
    w2 = np.ascontiguousarray(np.asarray(inputs["mlp_w2"], np.float32))
    b2 = np.asarray(inputs["mlp_b2"], np.float32).reshape(3, F, 1).copy()
    w3 = np.ascontiguousarray(np.asarray(inputs["mlp_w3"], np.float32) * inv_rbf).astype(md)
    b3 = (np.asarray(inputs["mlp_b3"], np.float32) * inv_rbf)
    b3_nonzero = bool(np.any(b3))
    b3 = b3.reshape(3, 1, RBF * F).astype(md)

    featp = np.concatenate(
        [
            np.asarray(inputs[f"feat{i}"], np.float32).transpose(2, 0, 1)
            for i in range(3)
        ],
        axis=0,
    ).astype(md)  # (9, A, F)
    sphp = np.concatenate(
        [
            np.asarray(inputs[f"sph{i}"], np.float32).transpose(2, 0, 1)
            for i in range(3)
        ],
        axis=0,
    ).astype(md)  # (9, A, T)
    rbfp = (
        np.asarray(inputs["radial_basis_vals"], np.float32)
        .transpose(2, 0, 1)
        .astype(md)
    )  # (RBF, A, T)

    idx = np.asarray(inputs["truncated_idx"]).astype(np.int64)
    first: dict = {}
    dup_pairs = []
    for t, v in enumerate(idx.tolist()):
        if v in first:
            dup_pairs.append((first[v], t))
        else:
            first[v] = t

    in_maps = []
    for c in range(NCORES):
        off = np.full(T, OOB, np.int32)
        lo, hi = c * NR, (c + 1) * NR
        for v, lt in first.items():
            if lo <= v < hi:
                off[lt] = v - lo
        offs2d = np.ascontiguousarray(off.reshape(T // 128, 128).T)
        ts = slice(c * TL, (c + 1) * TL)
        m = {
            "invT": invT,
            "w1": w1, "b1": b1, "w2": w2, "b2": b2, "w3": w3,
            "featp": featp,
            "sphp": np.ascontiguousarray(sphp[:, :, ts]),
            "rbfp": np.ascontiguousarray(rbfp[:, :, ts]),
            "offs": offs2d,
        }
        if b3_nonzero:
            m["b3"] = b3
        in_maps.append(m)
    return in_maps, tuple(dup_pairs), b3_nonzero


def _get_runner(dup_pairs, b3_nonzero, main_dtype, repeats=1):
    key = (dup_pairs, b3_nonzero, main_dtype, repeats)
    if key not in _CACHE:
        nc = _build_program(dup_pairs, b3_nonzero, main_dtype, repeats)
        _CACHE[key] = nc
    return _CACHE[key]


def run_on_hw(in_maps, nc):
    from concourse import bass_utils

    res = bass_utils.run_bass_kernel_spmd(
        nc, in_maps, core_ids=list(range(NCORES))
    )
    return res.results


def kernel(**inputs) -> np.ndarray:
    in_maps, dup_pairs, b3_nonzero = _prep(inputs, MAIN_DTYPE)
    nc = _get_runner(dup_pairs, b3_nonzero, MAIN_DTYPE)
    results = run_on_hw(in_maps, nc)
    return np.concatenate(
        [results[c]["out"] for c in range(NCORES)], axis=0
    )
